# revision 5
# baseline (speedup 1.0000x reference)
"""CrossAttention kernel for 8 Trainium2 NeuronCores (data-parallel over batch).

Math (per batch b, head h):
    q = query @ (64*Wq); k = key @ (64*Wk)          (fp8 DoubleRow matmuls)
    v = value @ Wv                                  (fp16)
    S^T[sk,sq] = k_h^T q_h                          (fp16, K=64)
    P = exp(S^T/(8*64*64)) * EM^T,  EM^T = exp(bias^T) * mask^T  (host-precomp)
    outT/denom via ones-augmented v:  [v_h | 1s]^T P -> [128, sq]
    attn_out^T[i,sq] = outT[0:64] / denom (rows 64.. = denom)    (DVE recip)
    out[sq,o] = attn_out^T.T @ Wo  (+ bo on host)

Engine split: Act = exp + q/k proj copies; DVE = PT mult (4 heads), recip,
attn mult, out/vaug copies; Pool(GpSimd, SBUF-only) = PT mult (4 heads).
"""
import os
import sys

import numpy as np

sys.path.insert(0, "/opt/trn_rl_repo")

import ml_dtypes

from concourse import bacc, bass, mybir, tile
from concourse.alu_op_type import AluOpType
from concourse.bass_utils import run_bass_kernel_spmd

B, S, D = 32, 512, 512
H, HD = 8, 64
NCORES = 8
BPC = B // NCORES  # batches per core
SCALE = 1.0 / np.sqrt(HD)
WSC = 64.0  # fp8 weight prescale (keeps W out of the e4m3 subnormal range)
EXP_SCALE = SCALE / (WSC * WSC)  # folded into the exp activation

FP16 = mybir.dt.float16
FP8 = mybir.dt.float8e4
F32 = mybir.dt.float32
NP_FP8 = ml_dtypes.float8_e4m3

POOL_PT_HEADS = (0, 1, 2, 4, 5, 6)  # sp=0 PT-mult heads on GpSimd

_last_results = None


def _build_nc():
    nc = bacc.Bacc("TRN2", target_bir_lowering=False, debug=False)

    # all inputs pre-laid-out on host as [.., 128, 4, S] (partition, chunk, s)
    qT8 = nc.dram_tensor("qT8", [BPC, 128, 4, S], FP8, kind="ExternalInput")
    kT8 = nc.dram_tensor("kT8", [BPC, 128, 4, S], FP8, kind="ExternalInput")
    vT = nc.dram_tensor("vT", [BPC, 128, 4, S], FP16, kind="ExternalInput")
    em = nc.dram_tensor("em", [BPC, 128, 4, S], FP16, kind="ExternalInput")
    wq8 = nc.dram_tensor("wq8", [128, 4, D], FP8, kind="ExternalInput")
    wk8 = nc.dram_tensor("wk8", [128, 4, D], FP8, kind="ExternalInput")
    wv = nc.dram_tensor("wv", [128, 4, D], FP16, kind="ExternalInput")
    wo = nc.dram_tensor("wo", [128, 4, D], FP16, kind="ExternalInput")
    out = nc.dram_tensor("out", [BPC, S, S], FP16, kind="ExternalOutput")

    Exp = mybir.ActivationFunctionType.Exp
    DR = mybir.MatmulPerfMode.DoubleRow

    with tile.TileContext(nc) as tc:
        with (
            tc.tile_pool(name="wpool", bufs=1) as wpool,
            tc.tile_pool(name="iop", bufs=4) as iop,
            tc.tile_pool(name="proj", bufs=2) as proj,
            tc.tile_pool(name="attn", bufs=3) as attn,
            tc.tile_pool(name="small", bufs=4) as small,
            tc.tile_pool(name="ps_proj", bufs=2, space="PSUM") as ps_proj,
            tc.tile_pool(name="ps_s", bufs=2, space="PSUM") as ps_s,
            tc.tile_pool(name="ps_o", bufs=2, space="PSUM") as ps_o,
        ):
            # weights resident: [d_part, d_chunk, out] layout
            w_sb = {}
            for name, drm, dt in (
                ("wq8", wq8, FP8),
                ("wk8", wk8, FP8),
                ("wv", wv, FP16),
                ("wo", wo, FP16),
            ):
                t = wpool.tile([128, 4, D], dt, tag=name, name=name)
                nc.sync.dma_start(t[:], drm.ap())
                w_sb[name] = t

            # vaug buffers persist outside pool rotation; the ones half is
            # written exactly once (denominator trick columns).
            vaugs = []
            for i in range(2):
                t = wpool.tile([128, 4, H, 2 * HD], FP16, tag=f"vaug{i}", name=f"vaug{i}")
                nc.vector.memset(t[:, :, :, 0:HD], 1.0)
                vaugs.append(t)

            # prefetch all batches' inputs (SBUF has room; removes per-batch
            # DMA waits from the critical path)
            ins = []
            for b in range(BPC):
                qT_sb = iop.tile([128, 4, S], FP8, tag="qT", name=f"qT{b}")
                kT_sb = iop.tile([128, 4, S], FP8, tag="kT", name=f"kT{b}")
                vT_sb = iop.tile([128, 4, S], FP16, tag="vT", name=f"vT{b}")
                em_sb = iop.tile([128, 4, S], FP16, tag="em", name=f"em{b}")
                nc.sync.dma_start(qT_sb[:], qT8[b])
                nc.sync.dma_start(kT_sb[:], kT8[b])
                nc.sync.dma_start(vT_sb[:], vT[b])
                nc.sync.dma_start(em_sb[:], em[b])
                ins.append((qT_sb, kT_sb, vT_sb, em_sb))

            for b in range(BPC):
                qT_sb, kT_sb, vT_sb, em_sb = ins[b]

                # ---- q/k projections (fp8 DoubleRow, K=256 per pass) ----
                qTp = proj.tile([128, 4, S], FP16, tag="qTp", name=f"qTp{b}")
                kTp = proj.tile([128, 4, S], FP16, tag="kTp", name=f"kTp{b}")
                for dst, w, src in ((qTp, w_sb["wq8"], qT_sb), (kTp, w_sb["wk8"], kT_sb)):
                    for it in range(4):
                        ps = ps_proj.tile([128, S], F32, tag="pp", name=f"pp{b}{it}")
                        for c in (0, 2):
                            nc.tensor.matmul(
                                ps[:],
                                w[:, c : c + 2, it * 128 : (it + 1) * 128],
                                src[:, c : c + 2, :],
                                start=(c == 0),
                                stop=(c == 2),
                                perf_mode=DR,
                            )
                        nc.scalar.copy(dst[:, it, :], ps[:])

                # v natural + ones columns: [sk_p, sk_c, h, 128]
                vaug = vaugs[b % 2]
                for st in range(4):
                    ps = ps_proj.tile([128, S], F32, tag="pp", name=f"ppv{b}{st}")
                    for c in range(4):
                        nc.tensor.matmul(
                            ps[:],
                            vT_sb[:, c, st * 128 : (st + 1) * 128],
                            w_sb["wv"][:, c, :],
                            start=(c == 0),
                            stop=(c == 3),
                        )
                    nc.vector.tensor_copy(
                        vaug[:, st, :, HD : 2 * HD],
                        ps[:].rearrange("p (h e) -> p h e", h=H),
                    )

                # ---- attention per head ----
                attn_oT = attn.tile([128, 4, S], FP16, tag="attn_oT", name=f"aot{b}")
                for h in range(H):
                    ic, po = h // 2, (h % 2) * 64
                    # S^T[sk,sq] = k_h^T q_h (fp16, K=64); exp batched per
                    # 2-chunk pair; PT mult batched per head
                    ex = small.tile([128, 4, S], FP16, tag="ex", name=f"ex{b}{h}")
                    PT = attn.tile([128, 4, S], FP16, tag="PT", name=f"PT{b}{h}")
                    for sp in range(2):
                        ps = ps_s.tile([128, 2 * S], F32, tag="sc", name=f"sc{b}{h}{sp}")
                        for j in range(2):
                            st = 2 * sp + j
                            nc.tensor.matmul(
                                ps[:, j * S : (j + 1) * S],
                                kTp[po : po + 64, ic, st * 128 : (st + 1) * 128],
                                qTp[po : po + 64, ic, :],
                                start=True,
                                stop=True,
                            )
                        nc.scalar.activation(
                            ex[:, 2 * sp : 2 * sp + 2, :],
                            ps[:].rearrange("p (j s) -> p j s", j=2),
                            Exp,
                            scale=float(EXP_SCALE),
                        )
                        pt_eng = (
                            nc.gpsimd
                            if sp == 0 and h in POOL_PT_HEADS
                            else nc.vector
                        )
                        pt_eng.tensor_tensor(
                            PT[:, 2 * sp : 2 * sp + 2, :],
                            ex[:, 2 * sp : 2 * sp + 2, :],
                            em_sb[:, 2 * sp : 2 * sp + 2, :],
                            op=AluOpType.mult,
                        )

                    # [v_h | 1s]^T @ P -> [128, sq]; rows 64.. all hold the denom
                    pso = ps_o.tile([128, S], F32, tag="ov", name=f"ov{b}{h}")
                    for c in range(4):
                        nc.tensor.matmul(
                            pso[:],
                            vaug[:, c, h, :],
                            PT[:, c, :],
                            start=(c == 0),
                            stop=(c == 3),
                        )
                    rd = small.tile([64, S], F32, tag="rd", name=f"rd{b}{h}")
                    nc.vector.reciprocal_approx_fast(rd[:], pso[0:HD, :])
                    nc.vector.tensor_tensor(
                        attn_oT[po : po + 64, ic, :],
                        pso[HD : 2 * HD, :],
                        rd[:],
                        op=AluOpType.mult,
                    )

                # ---- output projection; fp16 out ----
                for t in range(4):
                    pf = ps_o.tile([128, S], F32, tag="ov", name=f"pf{b}{t}")
                    for c in range(4):
                        nc.tensor.matmul(
                            pf[:],
                            attn_oT[:, c, t * 128 : (t + 1) * 128],
                            w_sb["wo"][:, c, :],
                            start=(c == 0),
                            stop=(c == 3),
                        )
                    osb = small.tile([128, S], FP16, tag="osb", name=f"osb{b}{t}")
                    (nc.scalar.copy if t % 2 == 0 else nc.vector.tensor_copy)(
                        osb[:], pf[:]
                    )
                    nc.sync.dma_start(out[b, t * 128 : (t + 1) * 128, :], osb[:])

    nc.compile()
    return nc


def _to_pcs(x, dtype):
    """[.., D, S] -> [.., 128, 4, S] with d = c*128 + p."""
    shp = x.shape[:-2]
    x = x.reshape(shp + (4, 128, S))
    perm = tuple(range(len(shp))) + (len(shp) + 1, len(shp), len(shp) + 2)
    return np.ascontiguousarray(x.transpose(perm).astype(dtype))


def _fp8(x):
    return np.clip(x, -240.0, 240.0).astype(NP_FP8)


def kernel(query, key, value, mask, Wq, Wk, Wv, Wo, bo, rel_pos_emb):
    global _last_results
    query = np.asarray(query)
    key = np.asarray(key)
    value = np.asarray(value)
    mask = np.asarray(mask)

    qT = query.transpose(0, 2, 1)
    kT = key.transpose(0, 2, 1)
    vT = value.transpose(0, 2, 1)
    ebT = np.exp(np.asarray(rel_pos_emb)[:S, :S].T.astype(np.float32))
    em = ebT[None, :, :] * mask.transpose(0, 2, 1).astype(np.float32)

    qT8 = _to_pcs(_fp8(qT), NP_FP8)
    kT8 = _to_pcs(_fp8(kT), NP_FP8)
    vT16 = _to_pcs(vT, np.float16)
    em16 = _to_pcs(em, np.float16)
    wq8 = _to_pcs(_fp8(np.asarray(Wq) * WSC), NP_FP8)
    wk8 = _to_pcs(_fp8(np.asarray(Wk) * WSC), NP_FP8)
    wv16 = _to_pcs(np.asarray(Wv), np.float16)
    wo16 = _to_pcs(np.asarray(Wo), np.float16)

    nc = _build_nc()
    in_maps = [
        {
            "qT8": qT8[i * BPC : (i + 1) * BPC],
            "kT8": kT8[i * BPC : (i + 1) * BPC],
            "vT": vT16[i * BPC : (i + 1) * BPC],
            "em": em16[i * BPC : (i + 1) * BPC],
            "wq8": wq8,
            "wk8": wk8,
            "wv": wv16,
            "wo": wo16,
        }
        for i in range(NCORES)
    ]
    trace = bool(int(os.environ.get("BASS_KERNEL_TRACE", "0")))
    res = run_bass_kernel_spmd(nc, in_maps, list(range(NCORES)), trace=trace)
    _last_results = res
    out = np.concatenate([res.results[i]["out"] for i in range(NCORES)], axis=0)
    return out.astype(np.float32) + np.asarray(bo)[None, None, :].astype(np.float32)
